# revision 36
# baseline (speedup 1.0000x reference)
"""Single attention head (B=8, S=2048, D=768, H=12) on 8 TRN2 NeuronCores.

Data-parallel over batch (1 element/core). Design:
  - Host prep is layout only: per-batch permutation packing masked-in keys
    first (key extent compacts 2048 -> T_pad ~ 1152), x transposed to
    [128, chunk, ko, 512] fp32 for contiguous DMA, weights packed
    [Wk | Wq/sqrt(H) | Wv] fp32 at 32-aligned columns, additive bias row.
  - QKV projection in ONE fp32r pass (fp32r matmuls stream at fp16 rate for
    moving dims >= 256, ~1.5e-4 relative error, fine for this near-one-hot
    softmax; measured end-to-end rel err ~8e-3 vs 2e-2 budget).
  - Pass A (row max, [s,t]): f32r 13-row matmul per s-tile from the same
    q/k tiles pass B uses; DVE reduce_max over 1024-wide PSUM slabs.
  - Pass B ([t,s]): f32r 14-row matmuls (12 q + bias + "-max" row); t-tile
    PAIRS run concurrently in PE row groups 0/64 (kTb and q tiles are
    replicated at partitions 64..78), sharing a [128,1024] PSUM tile so
    ACT exp runs 1024 wide.
  - PV: fp16, column-tiled 2 ways (M=16 at array cols 0/64); denominator
    rides along as a ones-column; DVE adds the two column-group partials.
  - Tile-granular dependency tracking forced per-chunk x / q tiles so DMA
    streams overlap compute; dummy matmuls on the weight tile keep the PE
    HAM-warm through the DMA-bound head; pass-A units interleave with
    B/PV pairs; replications ride idle DMA queues.
"""

import math
import os

import numpy as np

B, S, D, H = 8, 2048, 768, 12
N_CORES = 8
NCH = 4            # s chunks
SCH = S // NCH     # 512
BIAS_B = -1.0e8    # fp32 additive mask bias


def _build(nc_mod, T_pad):
    bass, mybir, tile, bacc = nc_mod
    f32 = mybir.dt.float32
    f32r = mybir.dt.float32r
    f16 = mybir.dt.float16
    AF = mybir.ActivationFunctionType
    X = mybir.AxisListType.X

    NT = T_pad // 128
    slabsA = [(o, min(512, T_pad - o)) for o in range(0, T_pad, 512)]
    last_cov = (T_pad - 1) // SCH

    nc = bacc.Bacc("TRN2", target_bir_lowering=False, debug=False,
                   num_devices=N_CORES)

    x_ext = nc.dram_tensor("x", [128, NCH * 6 * SCH], f32r,
                           kind="ExternalInput")
    w_ext = nc.dram_tensor("w", [128, 6 * 76], f32r, kind="ExternalInput")
    onesT_ext = nc.dram_tensor("onesT", [1, T_pad], f16, kind="ExternalInput")
    constB_ext = nc.dram_tensor("constB", [2, T_pad], f32r,
                                kind="ExternalInput")
    onesS_ext = nc.dram_tensor("onesS", [1, S], f32r, kind="ExternalInput")
    out_ext = nc.dram_tensor("out", [128, 256], f32, kind="ExternalOutput")

    from concourse.masks import make_identity

    with tile.TileContext(nc) as tc:
        with tc.tile_pool(name="sb", bufs=1) as sb, \
             tc.tile_pool(name="pp", bufs=4) as ppool, \
             tc.tile_pool(name="qv", bufs=1, space="PSUM") as qvp, \
             tc.tile_pool(name="ap", bufs=3, space="PSUM") as ap, \
             tc.tile_pool(name="bp", bufs=2, space="PSUM") as bp:

            xc = [sb.tile([128, 6, SCH], f32r, name=f"xc{c}")
                  for c in range(NCH)]
            xc0h = [sb.tile([128, 3, SCH], f32r, name=f"xc0h{h}")
                    for h in range(2)]
            w = sb.tile([128, 6, 76], f32r)
            # rows 0-11 k, 12 bias, 13 = -1; replicated at 64..78
            kTb = sb.tile([80, T_pad], f32r)
            # per-chunk q tiles: 0-11 q, 12 = 1, 13 = m; replica at 64..78
            rq = [sb.tile([80, SCH], f32r, name=f"rq{c}")
                  for c in range(NCH)]
            vaugT = sb.tile([32, T_pad], f16)   # 0-11 v, 12 = 1, rest 0
            vaug = sb.tile([128, NT, 16], f16)
            ident = sb.tile([128, 128], f32)
            ident16 = sb.tile([16, 16], f16)
            maxh = sb.tile([128, 16, 4], f32)
            maxc = sb.tile([128, 16], f32)
            negmT = sb.tile([4, 128], f32r)
            vcomb = sb.tile([32, S], f16)       # 0-12 combined out+denom
            vstage = sb.tile([16, S], f32)
            rec4 = sb.tile([128, 16], f32)
            outsb = sb.tile([128, 16, 16], f32)

            nc.gpsimd.memset(vaugT[:, :], 0.0)    # rows 13-31 stay 0
            nc.gpsimd.memset(vcomb[:, :], 0.0)    # rows 13-31 stay 0
            make_identity(nc, ident[:])
            make_identity(nc, ident16[:])

            xr0 = x_ext.ap().rearrange("p (c ko s) -> p c ko s",
                                       c=NCH, ko=6)
            nc.sync.dma_start(xc0h[0][:], xr0[:, 0, 0:3])
            nc.sync.dma_start(w[:], w_ext.ap().rearrange(
                "p (ko m) -> p ko m", ko=6))
            nc.sync.dma_start(xc0h[1][:], xr0[:, 0, 3:6])
            nc.gpsimd.dma_start(kTb[12:14, :], constB_ext.ap())
            nc.gpsimd.dma_start(kTb[76:78, :], constB_ext.ap())
            nc.gpsimd.dma_start(vaugT[12:13, :], onesT_ext.ap())
            for c in range(NCH):
                nc.gpsimd.dma_start(rq[c][12:13, :],
                                    onesS_ext.ap()[:, c * SCH:(c + 1) * SCH])
                nc.gpsimd.dma_start(rq[c][76:77, :],
                                    onesS_ext.ap()[:, c * SCH:(c + 1) * SCH])
            xr = x_ext.ap().rearrange("p (c ko s) -> p c ko s", c=NCH, ko=6)
            for c in range(1, NCH):
                nc.sync.dma_start(xc[c][:], xr[:, c])

            # ---- pass A / negm emitters (s-tile pairs on rows 0/64) ----
            def emit_A_slab(pr, si):
                st0, st1 = 2 * pr, 2 * pr + 1
                c = st0 // 4
                s0 = (st0 % 4) * 128
                s1 = (st1 % 4) * 128
                to, tw = slabsA[si]
                at0 = ap.tile([128, 512], f32, tag="pa512")
                at1 = ap.tile([128, 512], f32, tag="pa512")
                nc.tensor.matmul(
                    at0[:, 0:tw], rq[c][0:13, s0:s0 + 128],
                    kTb[0:13, to:to + tw], start=True, stop=True,
                    tile_position=(0, 0))
                nc.tensor.matmul(
                    at1[:, 0:tw], rq[c][64:77, s1:s1 + 128],
                    kTb[64:77, to:to + tw], start=True, stop=True,
                    tile_position=(64, 0))
                nc.vector.reduce_max(
                    maxh[:, st0, si:si + 1], at0[:, 0:tw], axis=X)
                nc.vector.reduce_max(
                    maxh[:, st1, si:si + 1], at1[:, 0:tw], axis=X)

            def emit_A_fin(pr):
                for st in (2 * pr, 2 * pr + 1):
                    nc.vector.reduce_max(
                        maxc[:, st:st + 1], maxh[:, st, 0:len(slabsA)],
                        axis=X)

            def emit_negm(c):
                c4 = slice(4 * c, 4 * c + 4)
                mt = ap.tile([128, 512], f32, tag="pa512")
                nc.tensor.transpose(mt[0:4, 0:128], maxc[:, c4], ident[:])
                nc.scalar.copy(negmT[:, :], mt[0:4, 0:128])
                for k in range(4):
                    nc.gpsimd.dma_start(rq[c][13:14, k * 128:(k + 1) * 128],
                                        negmT[k:k + 1, :])
                    nc.gpsimd.dma_start(rq[c][77:78, k * 128:(k + 1) * 128],
                                        negmT[k:k + 1, :])

            # ---- QKV projection (fp32r), one pass, DMA interleaved ----
            def emit_qkv(c):
                qkv = qvp.tile([76, SCH], f32, tag="qv", name=f"qkv{c}")
                for ko in range(6):
                    xin = (xc0h[ko // 3][:, ko % 3, :] if c == 0
                           else xc[c][:, ko, :])
                    nc.tensor.matmul(qkv[:, :], w[:, ko, :], xin,
                                     start=(ko == 0), stop=(ko == 5))
                nc.scalar.copy(rq[c][0:12, :], qkv[32:44, :])
                nc.gpsimd.dma_start(rq[c][64:77, :], rq[c][0:13, :])
                if c * SCH < T_pad:
                    t0 = c * SCH
                    t1 = min((c + 1) * SCH, T_pad)
                    tsl = slice(0, t1 - t0)
                    ts = slice(t0, t1)
                    nc.scalar.copy(kTb[0:12, ts], qkv[0:12, tsl])
                    nc.gpsimd.dma_start(kTb[64:76, ts], kTb[0:12, ts])
                    nc.scalar.copy(vaugT[0:12, ts], qkv[64:76, tsl])

            emit_qkv(0)
            emit_qkv(1)
            emit_qkv(2)
            # chunk 0-1 row maxes; chunks 2-3 ride the main loop as fillers
            nsl01 = min(2, len(slabsA))
            for si in range(nsl01):
                for pr in range(4):
                    emit_A_slab(pr, si)
            emit_qkv(3)
            for si in range(nsl01, len(slabsA)):
                for pr in range(4):
                    emit_A_slab(pr, si)
            for pr in range(2):
                emit_A_fin(pr)
            emit_negm(0)
            for pr in range(2, 4):
                emit_A_fin(pr)
            emit_negm(1)

            # ---- attention main loop ----
            npair = (NT + 1) // 2
            g_last = {0: 2 * (npair - 1)}
            g_last[1] = 2 * ((NT - 2) // 2) + 1 if NT >= 2 else -1

            def emit_out_st(st):
                ot = ap.tile([128, 512], f16, name=f"ot{st}", tag="pa512")
                nc.tensor.transpose(
                    ot[:, 0:16], vcomb[0:16, st * 128:(st + 1) * 128],
                    ident16[:])
                nc.vector.reciprocal(rec4[:, st:st + 1], ot[:, 12:13])
                nc.vector.tensor_scalar_mul(
                    outsb[:, st, 0:12], ot[:, 0:12], rec4[:, st:st + 1])

            pcur = {}

            def emit_B_pair(c, jp, vacc):
                j0, j1 = 2 * jp, 2 * jp + 1
                width = 1024 if j1 < NT else 512
                bt = bp.tile([128, 1024], f32, tag="b")
                nc.tensor.matmul(
                    bt[:, 0:512], kTb[0:14, j0 * 128:(j0 + 1) * 128],
                    rq[c][0:14, :], start=True, stop=True,
                    tile_position=(0, 0))
                if j1 < NT:
                    nc.tensor.matmul(
                        bt[:, 512:1024],
                        kTb[64:78, j1 * 128:(j1 + 1) * 128],
                        rq[c][64:78, :], start=True, stop=True,
                        tile_position=(64, 0))
                p = ppool.tile([128, 1024], f16, tag="p")
                nc.scalar.activation(p[:, 0:width], bt[:, 0:width], AF.Exp)
                pcur[jp] = p

            def emit_PV_pair(c, jp, vacc):
                j0, j1 = 2 * jp, 2 * jp + 1
                p = pcur.pop(jp)
                nc.tensor.matmul(
                    vacc[0:16, :], vaug[:, j0, 0:16], p[:, 0:512],
                    start=(j0 == 0), stop=(j0 == g_last[0]),
                    tile_position=(0, 0))
                if j1 < NT:
                    nc.tensor.matmul(
                        vacc[64:80, :], vaug[:, j1, 0:16], p[:, 512:1024],
                        start=(j1 == 1), stop=(j1 == g_last[1]),
                        tile_position=(0, 64))

            def emit_vt(j):
                vt = ap.tile([128, 512], f16, name=f"vt{j}", tag="pa512")
                nc.tensor.transpose(
                    vt[:, 0:16], vaugT[0:16, j * 128:(j + 1) * 128],
                    ident16[:])
                nc.scalar.copy(vaug[:, j, 0:16], vt[:, 0:16])

            # filler work queues per chunk: remaining pass-A units, negms,
            # v transposes, and out-stage units of earlier chunks
            fillq = {c: [] for c in range(NCH)}
            for j in range(NT):
                fillq[0].append(lambda j=j: emit_vt(j))
            for pr in (4, 5):
                for si in range(len(slabsA)):
                    fillq[0].append(lambda pr=pr, si=si: emit_A_slab(pr, si))
            fillq[0].append(lambda: (emit_A_fin(4), emit_A_fin(5),
                                     emit_negm(2)))
            for pr in (6, 7):
                for si in range(len(slabsA)):
                    fillq[1].append(lambda pr=pr, si=si: emit_A_slab(pr, si))
            fillq[1].append(lambda: (emit_A_fin(6), emit_A_fin(7),
                                     emit_negm(3)))
            for c in range(1, NCH):
                for st in range(4 * (c - 1), 4 * c):
                    fillq[c].append(lambda st=st: emit_out_st(st))

            for c in range(NCH):
                cs = slice(c * SCH, (c + 1) * SCH)
                vacc = qvp.tile([96, SCH], f32, tag="qv", name=f"vacc{c}")
                fillers = fillq[c]
                nslot = npair + 2
                for jp in range(nslot):
                    if jp < npair:
                        emit_B_pair(c, jp, vacc)
                    take = (len(fillers) + nslot - 1 - jp) // (nslot - jp) \
                        if fillers else 0
                    for _ in range(take):
                        fillers.pop(0)()
                    if jp >= 2:
                        emit_PV_pair(c, jp - 2, vacc)
                nc.scalar.copy(vstage[0:16, cs], vacc[64:80, :])
                nc.vector.tensor_add(vcomb[0:16, cs], vacc[0:16, :],
                                     vstage[0:16, cs])
            for st in range(12, 16):
                emit_out_st(st)

            outr = out_ext.ap().rearrange("p (a b) -> p a b", a=16)
            for c in range(NCH):
                nc.sync.dma_start(outr[:, 4 * c:4 * c + 4, :],
                                  outsb[:, 4 * c:4 * c + 4, :])

    nc.compile()
    return nc


def kernel(x, mask, key_weight, query_weight, value_weight):
    import concourse.bass as bass
    import concourse.mybir as mybir
    import concourse.tile as tile
    from concourse import bacc, bass_utils

    x = np.asarray(x, dtype=np.float32)
    mask = np.asarray(mask)
    wk = np.asarray(key_weight, dtype=np.float32)
    wq = np.asarray(query_weight, dtype=np.float32)
    wv = np.asarray(value_weight, dtype=np.float32)

    w2 = np.zeros((D, 76), dtype=np.float32)
    w2[:, 0:12] = wk
    w2[:, 32:44] = wq / math.sqrt(H)
    w2[:, 64:76] = wv
    w_dev = np.ascontiguousarray(
        w2.reshape(6, 128, 76).transpose(1, 0, 2)).reshape(128, 6 * 76)

    perms, nbs = [], []
    for b in range(B):
        m = mask[b, 0].astype(np.int64)
        perm = np.argsort(1 - m, kind="stable")
        perms.append(perm)
        nbs.append(int(m.sum()))
    T_pad = max(128, int(np.ceil(max(max(nbs), 1) / 128.0)) * 128)
    T_pad = min(T_pad, S)

    in_maps = []
    for b in range(B):
        xp = x[b][perms[b]]                    # [S, D]
        xp = xp.reshape(NCH, SCH, 6, 128)      # [c, s, ko, p]
        x_dev = np.ascontiguousarray(
            xp.transpose(3, 0, 2, 1)).reshape(128, NCH * 6 * SCH)
        constB = np.zeros((2, T_pad), dtype=np.float32)
        constB[0, nbs[b]:] = BIAS_B
        constB[1, :] = -1.0
        in_maps.append({"x": x_dev, "w": w_dev,
                        "constB": constB,
                        "onesS": np.ones((1, S), dtype=np.float32),
                        "onesT": np.ones((1, T_pad), dtype=np.float16)})

    import time as _time
    _t0 = _time.time()
    print(f"[kernel] building graph, T_pad={T_pad}", flush=True)
    nc = _build((bass, mybir, tile, bacc), T_pad)
    print(f"[kernel] graph+bacc compile done in {_time.time() - _t0:.1f}s",
          flush=True)

    trace = os.environ.get("BASS_KERNEL_TRACE", "0") == "1"
    if trace:
        import sys
        import types
        from trn_agent_boot.trn_boot import _ntff_profile_via_ctypes
        hook = _ntff_profile_via_ctypes("/opt/axon/libaxon_pjrt.so")
        m = types.ModuleType("antenv.axon_hooks")
        m.get_axon_ntff_profile_hook = lambda: hook
        sys.modules["antenv.axon_hooks"] = m
        bass_utils.upload_artifacts = lambda tmpdir: "local://" + tmpdir

    res = bass_utils.run_bass_kernel_spmd(
        nc, in_maps, core_ids=list(range(N_CORES)), trace=trace)
    if trace:
        print(f"HW exec time: {res.exec_time_ns} ns", flush=True)

    out = np.empty((B, S, H), dtype=np.float32)
    for b in range(B):
        o = res.results[b]["out"].reshape(128, 16, 16)[:, :, :H]
        out[b, perms[b], :] = o.transpose(1, 0, 2).reshape(S, H)
    return out
